# revision 1
# baseline (speedup 1.0000x reference)
"""Cross-attention kernel for Trainium2 (8 NeuronCores, data-parallel over batch).

Computation (per batch element b, H=16 heads, D=64 head dim, C=1024):
    Q  = x_b @ q_w                      [1024, 1024]
    K  = context @ kv_w[:, :1024]       [2048, 1024]
    V  = context @ kv_w[:, 1024:]       [2048, 1024]
    S_h = (Q_h K_h^T) / sqrt(D)         [1024, 2048] per head
    P_h = softmax(S_h, axis=-1)
    O_h = P_h V_h                       [1024, 64]
    out = concat_h(O_h) @ proj_w + proj_b

Sharding: data parallel -- core i computes batch element i. Host->device
dispatch cost in this environment is dominated by input bytes AND buffer
count, so each core receives exactly ONE bf16 input tensor: its batch
element stacked with a 1/8 row-shard of every replicated tensor.
On-device NeuronLink AllGathers reassemble what each core needs:
  - q/k/v weight shards are gathered up front (c1) and proj_w+bias later
    (c2, consumed last);
  - the context shard is NEVER gathered: each core projects its local 256
    context rows to K^T/V shards (1/8 of the KV-projection FLOPs) and the
    K^T / V results are gathered instead, straight into the DRAM layout
    phase E consumes.  The K^T gather is split in two so phase E starts
    after the first half; the proj_w gather is issued last (consumed last).
    This also deletes the full-context PE transposes and the K/V DRAM
    bounce round-trip of the replicated design.
Per-core shipped input is 3.7 MB (1 buffer); output returns bf16.

Per-core blob layout ([1793, 1024] bf16 rows):
    0:1024    x_b
    1024:1152 q_w rows     [128 i : 128(i+1)]
    1152:1280 kv_w[:, :C]  rows (K weight)
    1280:1408 kv_w[:, C:]  rows (V weight)
    1408:1536 proj_w rows
    1536:1537 proj_b
    1537:1793 context rows [256 i : 256(i+1)]  (stays local)

Device pipeline (bf16 operands, fp32 PSUM accumulation):
  G1. bounce blob[1024:1408] -> c1 AllGather (qw|kw|vw per-core chunks);
      bounce blob[1408:1537] for the late pw|pb AllGather.
  A.  x -> x_T [c, q] and ctx_s -> ctx_sT [c, 256] via PE transposes (local).
  D.  KT_s [hd, 256] = kv_w-stationary @ ctx_sT; V_s [256, hd] =
      ctx_sT-stationary @ kv_w -> DRAM bounces -> AllGathers into
      ktg_a/ktg_c [8, 4, 128, 256] and vg [16, 128, 1024] (global kv
      order), then the pw|pb gather last.
  B.  QT [hd, q] = q_w-stationary @ x_T  (transposed-output projection),
      overlapping the c3/c4 gathers.
  E.  per head pair (heads 2p/2p+1 row-packed at partitions 0-63/64-127):
      S_T[k, q] = KT-slice stationary @ QT moving; exp(S*scale) fused on ACT
      (no max subtraction -- scores are ~N(0,1) so exp is safe in f32);
      P@V'-accumulation with V' = [V | ones] yields O^T[d, q] plus the
      softmax denominator in one PSUM group; 1/denom is broadcast across
      partitions via a DRAM bounce and applied by DVE during PSUM eviction.
  F.  out[q, c] = O^T-stationary @ proj_w + proj_b, natural layout.
"""

import sys

if "/opt/trn_rl_repo" not in sys.path:
    sys.path.insert(0, "/opt/trn_rl_repo")

import numpy as np
import ml_dtypes

import concourse.bass as bass
import concourse.tile as tile
from concourse import bacc, mybir
from concourse.bass_utils import run_bass_kernel_spmd
from concourse.masks import make_identity

F32 = mybir.dt.float32
BF16_NP = ml_dtypes.bfloat16

B = 8
NQ = 1024
NKV = 2048
C = 1024
H = 16
D = 64
P = 128
SCALE = D ** -0.5

CS = C // B        # weight row-shard per core (128)
KVS = NKV // B     # context row-shard per core (256)

# blob row offsets (in units of 1024-element rows)
R_X = 0
R_QW = NQ                  # 1024   } c1 gather region (384 rows)
R_KW = R_QW + CS           # 1152
R_VW = R_KW + CS           # 1280
R_PW = R_VW + CS           # 1408   } c2 gather region (129 rows)
R_PB = R_PW + CS           # 1536
R_CTX = R_PB + 1           # 1537   } local
R_BLOB = R_CTX + KVS       # 1793
R_G1 = 3 * CS              # rows per core in c1 (384)
R_G2 = CS + 1              # rows per core in c2 (129)

# Matmul/storage dtype on device: bf16 operands, fp32 PSUM accumulation.
MDT = mybir.dt.bfloat16

import os
REPEAT = int(os.environ.get("K_REPEAT", "1"))

GROUPS = [list(range(B))]


def _build_kernel():
    nc = bacc.Bacc("TRN2", target_bir_lowering=False, debug=False, num_devices=B)

    blob_in = nc.dram_tensor("blob", [R_BLOB, C], MDT, kind="ExternalInput").ap()
    out_d = nc.dram_tensor("out", [NQ, C], MDT, kind="ExternalOutput").ap()

    with tile.TileContext(nc) as tc:
        _emit(nc, tc, blob_in, out_d)

    nc.compile()
    return nc


def _emit(nc, tc, blob_in, out_d):
    from contextlib import ExitStack

    ctx = ExitStack()
    with ctx:
        dram = ctx.enter_context(tc.tile_pool(name="dram", bufs=1, space="DRAM"))
        rdram = ctx.enter_context(tc.tile_pool(name="rdram", bufs=4, space="DRAM"))

        # collective bounce-ins (internal DRAM) + gathered outputs (Shared)
        g1_b = dram.tile([R_G1, C], MDT)
        g2_b = dram.tile([R_G2, C], MDT)
        kts_a = dram.tile([H // 4, P, KVS], MDT)     # KT shard planes 0-3
        kts_c = dram.tile([H // 4, P, KVS], MDT)     # KT shard planes 4-7
        vs_b = dram.tile([KVS // P, P, C], MDT)      # V shard  [256, hd]
        g1 = nc.dram_tensor("g1", [B * R_G1, C], MDT, addr_space="Shared").ap()
        g2 = nc.dram_tensor("g2", [B * R_G2, C], MDT, addr_space="Shared").ap()
        ktg_a = nc.dram_tensor("ktg_a", [B, H // 4, P, KVS], MDT,
                               addr_space="Shared").ap()
        ktg_c = nc.dram_tensor("ktg_c", [B, H // 4, P, KVS], MDT,
                               addr_space="Shared").ap()
        vg = nc.dram_tensor("vg", [NKV // P, P, C], MDT,
                            addr_space="Shared").ap()

        for _rep in range(REPEAT):
            _emit_body(nc, tc, _rep, rdram, g1_b, g2_b, (kts_a, kts_c), vs_b,
                       g1, g2, (ktg_a, ktg_c), vg, blob_in, out_d)


def _emit_body(nc, tc, rep, rdram, g1_b, g2_b, kts_b, vs_b, g1, g2, ktg, vg,
               blob_in, out_d):
    from contextlib import ExitStack
    ctx = ExitStack()
    kts_a, kts_c = kts_b
    ktg_a, ktg_c = ktg
    with ctx:
        # ---------------- Phase G1: gather q/k/v weights, then proj -------
        nc.gpsimd.dma_start(g1_b[:], blob_in[R_QW:R_PW, :])
        nc.gpsimd.collective_compute(
            "AllGather", mybir.AluOpType.bypass, replica_groups=GROUPS,
            ins=[g1_b.opt()], outs=[g1.opt()])
        nc.gpsimd.dma_start(g2_b[:], blob_in[R_PW:R_CTX, :])

        persist = ctx.enter_context(tc.tile_pool(name=f"persist{rep}", bufs=1))
        qt_sb = persist.tile([P, C // P, NQ], MDT)      # QT [hd, q]: 16KB/p
        ident_f = persist.tile([P, P], F32)
        make_identity(nc, ident_f)
        ident = persist.tile([P, P], MDT)
        nc.vector.tensor_copy(ident, ident_f)

        # ---------------- Phase A: local transposes (x, ctx shard) --------
        with tc.tile_pool(name="xab", bufs=1) as xab, \
             tc.tile_pool(name="ldA", bufs=3) as ldA, \
             tc.tile_pool(name="wq", bufs=8) as wqp, \
             tc.tile_pool(name="wkv", bufs=8) as wkv, \
             tc.tile_pool(name="ev", bufs=6) as ev, \
             tc.tile_pool(name="pst", bufs=2, space="PSUM") as pst, \
             tc.tile_pool(name="psp", bufs=4, space="PSUM") as psp:
            ctx_st = xab.tile([P, C // P, KVS], MDT)    # ctx_s^T: 4KB/p
            for kt in range(KVS // P):
                ca = ldA.tile([P, C], MDT, tag="xa")
                nc.sync.dma_start(
                    ca, blob_in[R_CTX + kt * P:R_CTX + (kt + 1) * P, :])
                for ct in range(C // P):
                    ps = pst.tile([P, P], MDT)
                    nc.tensor.transpose(ps, ca[:, ct * P:(ct + 1) * P], ident)
                    nc.vector.tensor_copy(ctx_st[:, ct, kt * P:(kt + 1) * P], ps)

            x_t = xab.tile([P, C // P, NQ], MDT)        # x^T [c, q]: 16KB/p
            for qt in range(NQ // P):
                xa = ldA.tile([P, C], MDT, tag="xa")
                nc.sync.dma_start(xa, blob_in[qt * P:(qt + 1) * P, :])
                for ct in range(C // P):
                    ps = pst.tile([P, P], MDT)
                    nc.tensor.transpose(ps, xa[:, ct * P:(ct + 1) * P], ident)
                    nc.vector.tensor_copy(x_t[:, ct, qt * P:(qt + 1) * P], ps)

            # ---------------- Phase D: local KT/V shards + gathers --------
            kvw_k = []
            kvw_v = []
            for c in range(C // P):
                wk = wkv.tile([P, C], MDT, tag="wk")
                nc.sync.dma_start(wk, g1[c * R_G1 + CS:c * R_G1 + 2 * CS, :])
                kvw_k.append(wk)
                wv = wkv.tile([P, C], MDT, tag="wv")
                nc.sync.dma_start(wv, g1[c * R_G1 + 2 * CS:c * R_G1 + 3 * CS, :])
                kvw_v.append(wv)

            # KT_s [hd-plane, 256]: kvw_k stationary, ctx_sT moving.
            # Two half-gathers so phase E can start after the first one.
            for half, (ktsh, ktgh) in enumerate(((kts_a, ktg_a),
                                                 (kts_c, ktg_c))):
                for cp in range(C // P // 2):
                    colt = half * (C // P // 2) + cp
                    ps = psp.tile([P, KVS], F32)
                    for c in range(C // P):
                        nc.tensor.matmul(
                            ps,
                            kvw_k[c][:, colt * P:(colt + 1) * P],
                            ctx_st[:, c, :],
                            start=(c == 0), stop=(c == C // P - 1))
                    st = ev.tile([P, KVS], MDT, tag="kst")
                    nc.vector.tensor_copy(st, ps)
                    nc.sync.dma_start(ktsh[cp], st)
                nc.gpsimd.collective_compute(
                    "AllGather", mybir.AluOpType.bypass, replica_groups=GROUPS,
                    ins=[ktsh.opt()], outs=[ktgh.opt()])

            # V_s [kv-tile, hd]: ctx_sT stationary, kvw_v moving
            for vt in range(KVS // P):
                for hdch in range(C // 512):
                    ps = psp.tile([P, 512], F32)
                    for c in range(C // P):
                        nc.tensor.matmul(
                            ps,
                            ctx_st[:, c, vt * P:(vt + 1) * P],
                            kvw_v[c][:, hdch * 512:(hdch + 1) * 512],
                            start=(c == 0), stop=(c == C // P - 1))
                    st = ev.tile([P, 512], MDT, tag="vst")
                    nc.vector.tensor_copy(st, ps)
                    nc.sync.dma_start(
                        vs_b[vt, :, hdch * 512:(hdch + 1) * 512], st)
            nc.gpsimd.collective_compute(
                "AllGather", mybir.AluOpType.bypass, replica_groups=GROUPS,
                ins=[vs_b.opt()], outs=[vg.opt()])
            nc.gpsimd.collective_compute(
                "AllGather", mybir.AluOpType.bypass, replica_groups=GROUPS,
                ins=[g2_b.opt()], outs=[g2.opt()])

            # ---------------- Phase B: QT (overlaps the KT/V gathers) -----
            qw_sb = []
            for c in range(C // P):
                w = wqp.tile([P, C], MDT, tag="qw")
                nc.sync.dma_start(w, g1[c * R_G1:c * R_G1 + CS, :])
                qw_sb.append(w)
            for mt in range(C // P):
                for qch in range(NQ // 512):
                    ps = psp.tile([P, 512], F32)
                    for c in range(C // P):
                        nc.tensor.matmul(
                            ps,
                            qw_sb[c][:, mt * P:(mt + 1) * P],
                            x_t[:, c, qch * 512:(qch + 1) * 512],
                            start=(c == 0), stop=(c == C // P - 1))
                    nc.vector.tensor_copy(qt_sb[:, mt, qch * 512:(qch + 1) * 512], ps)

        # ---------------- Phase E: attention per head pair ----------------
        o_pool = ctx.enter_context(tc.tile_pool(name=f"o_pool{rep}", bufs=1))
        o_sb = o_pool.tile([P, C // P, NQ], MDT)        # O^T [hd, q]: 16KB/p

        NKT = NKV // P  # 16 k tiles
        ones_t = o_pool.tile([P, NKT, 2, 1], F32)
        nc.vector.memset(ones_t, 1.0)
        with tc.tile_pool(name="kv_e", bufs=2) as kv_e, \
             tc.tile_pool(name="epool", bufs=NKT + 2) as epool, \
             tc.tile_pool(name="rp", bufs=3) as rp, \
             tc.tile_pool(name="ps_s", bufs=2, space="PSUM") as ps_s, \
             tc.tile_pool(name="ps_pv", bufs=4, space="PSUM") as ps_pv:
            for hp in range(H // 2):
                ktp = kv_e.tile([P, NKV], MDT, tag="ktp")     # 4KB/p
                ktgh = ktg_a if hp < H // 4 else ktg_c
                nc.sync.dma_start(
                    ktp, ktgh[:, hp % (H // 4), :, :].transpose([1, 0, 2]))
                vp = kv_e.tile([P, NKT, 2, D + 1], MDT, tag="vp")  # 4.1KB/p
                nc.vector.tensor_copy(vp[:, :, :, D:D + 1], ones_t)
                for hh in range(2):
                    h = 2 * hp + hh
                    nc.sync.dma_start(
                        vp[:, :, hh, 0:D],
                        vg[:, :, h * D:(h + 1) * D].transpose([1, 0, 2]))

                for qh in range(NQ // 512):
                    qs = slice(qh * 512, (qh + 1) * 512)
                    e_tiles = [[None] * (NKT // 2) for _ in range(2)]
                    for j2 in range(NKT // 2):
                        for hh in range(2):
                            ps = ps_s.tile([P, 2, 512], F32)
                            for j in range(2):
                                kt = 2 * j2 + j
                                nc.tensor.matmul(
                                    ps[:, j, :],
                                    ktp[hh * D:(hh + 1) * D,
                                        kt * P:(kt + 1) * P],
                                    qt_sb[hh * D:(hh + 1) * D, hp, qs],
                                    start=True, stop=True)
                            et = epool.tile([P, 2, 512], MDT, tag="e")
                            nc.scalar.activation(
                                et, ps, mybir.ActivationFunctionType.Exp,
                                scale=SCALE)
                            e_tiles[hh][j2] = et
                    for hh in range(2):
                        pso = ps_pv.tile([P, 512], F32)
                        for j2 in range(NKT // 2):
                            for j in range(2):
                                kt = 2 * j2 + j
                                nc.tensor.matmul(
                                    pso[0:D + 1, :],
                                    vp[:, kt, hh, :],
                                    e_tiles[hh][j2][:, j, :],
                                    start=(kt == 0), stop=(kt == NKT - 1))
                        # reciprocal of the softmax denominator (row 64),
                        # broadcast to 64 partitions via a DRAM bounce
                        # (SBUF-source partition-step-0 DMA is illegal).
                        rrow = rp.tile([P, 512], F32, tag="rrow")
                        nc.vector.reciprocal(rrow[D:D + 1, :], pso[D:D + 1, :])
                        rd = rdram.tile([1, 512], F32, tag="rd")
                        nc.sync.dma_start(rd, rrow[D:D + 1, :])
                        rbc = rp.tile([D, 512], F32, tag="rbc")
                        nc.sync.dma_start(rbc, rd.partition_broadcast(D))
                        if hh == 0:
                            nc.vector.tensor_mul(
                                o_sb[0:D, hp, qs], pso[0:D, :], rbc)
                        else:
                            ost = rp.tile([D, 512], MDT, tag="ost")
                            nc.vector.tensor_mul(ost, pso[0:D, :], rbc)
                            nc.sync.dma_start(o_sb[D:2 * D, hp, qs], ost)

        # ---------------- Phase F: final projection ----------------
        with tc.tile_pool(name="wp", bufs=9) as wpp, \
             tc.tile_pool(name="fin", bufs=3) as finp, \
             tc.tile_pool(name="psp", bufs=4, space="PSUM") as psp:
            bias_bc = wpp.tile([P, C], MDT, tag="bias")
            pb2 = g2[CS:CS + 1, :]  # [1, C] bias from core 0's chunk
            nc.sync.dma_start(bias_bc, pb2.partition_broadcast(P))
            pw_sb = []
            for hc in range(C // P):
                w = wpp.tile([P, C], MDT, tag="pw")
                nc.sync.dma_start(w, g2[hc * R_G2:hc * R_G2 + CS, :])
                pw_sb.append(w)
            for qt in range(NQ // P):
                for cch in range(C // 512):
                    ps = psp.tile([P, 512], F32)
                    for hc in range(C // P):
                        nc.tensor.matmul(
                            ps,
                            o_sb[:, hc, qt * P:(qt + 1) * P],
                            pw_sb[hc][:, cch * 512:(cch + 1) * 512],
                            start=(hc == 0), stop=(hc == C // P - 1))
                    ft = finp.tile([P, 512], MDT, tag="fin")
                    nc.vector.tensor_add(ft, ps, bias_bc[:, cch * 512:(cch + 1) * 512])
                    nc.sync.dma_start(
                        out_d[qt * P:(qt + 1) * P, cch * 512:(cch + 1) * 512], ft)


_CACHED_NC = None


def _get_nc():
    global _CACHED_NC
    if _CACHED_NC is None:
        _CACHED_NC = _build_kernel()
    return _CACHED_NC


_RUNNER = None


def _get_runner():
    """Jit the 8-core shard_map execute ONCE and reuse it across kernel()
    calls (run_bass_kernel_spmd builds a fresh closure per call, paying
    ~1.3 s of retrace/recompile each time). Mirrors
    bass2jax.run_bass_via_pjrt's multi-core path, without donation so the
    callable is reusable."""
    global _RUNNER
    if _RUNNER is None:
        import jax
        from jax.sharding import Mesh, PartitionSpec
        from jax.experimental.shard_map import shard_map
        from concourse.bass2jax import (
            _bass_exec_p, install_neuronx_cc_hook, partition_id_tensor)

        nc = _get_nc()
        install_neuronx_cc_hook()
        partition_name = (nc.partition_id_tensor.name
                          if nc.partition_id_tensor else None)
        in_names, out_names, out_avals = [], [], []
        for alloc in nc.m.functions[0].allocations:
            if not isinstance(alloc, mybir.MemoryLocationSet):
                continue
            name = alloc.memorylocations[0].name
            if alloc.kind == "ExternalInput":
                if name != partition_name:
                    in_names.append(name)
            elif alloc.kind == "ExternalOutput":
                out_names.append(name)
                out_avals.append(jax.core.ShapedArray(
                    tuple(alloc.tensor_shape), mybir.dt.np(alloc.dtype)))
        all_in = list(in_names) + list(out_names)
        if partition_name is not None:
            all_in.append(partition_name)

        def _body(*args):
            operands = list(args)
            if partition_name is not None:
                operands.append(partition_id_tensor())
            return tuple(_bass_exec_p.bind(
                *operands, out_avals=tuple(out_avals), in_names=tuple(all_in),
                out_names=tuple(out_names), lowering_input_output_aliases=(),
                sim_require_finite=True, sim_require_nnan=True, nc=nc))

        devices = jax.devices()[:B]
        assert len(devices) == B
        mesh = Mesh(np.asarray(devices), ("core",))
        nio = len(in_names) + len(out_names)
        fn = jax.jit(
            shard_map(_body, mesh=mesh, in_specs=(PartitionSpec("core"),) * nio,
                      out_specs=(PartitionSpec("core"),) * len(out_names),
                      check_rep=False),
            keep_unused=True)
        _RUNNER = (fn, in_names, out_names, out_avals)
    return _RUNNER


def make_in_maps(x, context, q_w, kv_w, proj_w, proj_b):
    """Pack per-core blobs: batch element + 1/8 shard of replicated tensors."""
    x = np.asarray(x)
    context = np.asarray(context)
    q_w = np.asarray(q_w)
    kv_w = np.asarray(kv_w)
    proj_w = np.asarray(proj_w)
    proj_b = np.asarray(proj_b)
    in_maps = []
    for i in range(B):
        blob = np.empty((R_BLOB, C), dtype=BF16_NP)
        blob[R_X:R_QW] = x[i]
        blob[R_QW:R_KW] = q_w[i * CS:(i + 1) * CS]
        blob[R_KW:R_VW] = kv_w[i * CS:(i + 1) * CS, :C]
        blob[R_VW:R_PW] = kv_w[i * CS:(i + 1) * CS, C:]
        blob[R_PW:R_PB] = proj_w[i * CS:(i + 1) * CS]
        blob[R_PB] = proj_b
        blob[R_CTX:R_BLOB] = context[i * KVS:(i + 1) * KVS]
        in_maps.append({"blob": blob})
    return in_maps


def _run_cached(in_maps):
    fn, in_names, out_names, out_avals = _get_runner()
    concat = [np.concatenate([np.asarray(in_maps[c][n]) for c in range(B)],
                             axis=0) for n in in_names]
    concat += [np.zeros((B * av.shape[0], *av.shape[1:]), av.dtype)
               for av in out_avals]
    outs = fn(*concat)
    i = out_names.index("out")
    return np.asarray(outs[i]).reshape(B, NQ, C)


def kernel(x, context, q_w, kv_w, proj_w, proj_b):
    in_maps = make_in_maps(x, context, q_w, kv_w, proj_w, proj_b)
    last_err = None
    for _attempt in range(3):
        try:
            out = _run_cached(in_maps)
            break
        except Exception as e:
            last_err = e
            global _RUNNER
            _RUNNER = None  # rebuild the runner on retry
            import time as _time
            _time.sleep(2.0)
    else:
        # final fallback: the stock (per-call jit) dispatch path
        res = run_bass_kernel_spmd(_get_nc(), in_maps,
                                   core_ids=list(range(B)))
        out = np.stack([np.asarray(res.results[i]["out"]) for i in range(B)],
                       axis=0)
    return out.astype(np.float32)

